# revision 1
# baseline (speedup 1.0000x reference)
"""Trainium2 Bass kernel for nn_LocalExperts (MoE grouped FFN).

out[e] = relu(x[e] @ wi[e]) @ wo[e]   for e in 0..7

Expert-parallel over 8 NeuronCores: core e computes expert e's FFN.
Per-core work: x [8192, 512] f32, wi [512, 2048], wo [2048, 512]
  GEMM1: hT[f, m] = wi[d, f].T @ xT[d, m]  (accumulate over 4 d-chunks)
  relu (ScalarE) -> hT in SBUF as float32r
  GEMM2: out[m, d] = hT[f, m].T @ wo[f, d] (accumulate over 16 f-chunks)
x is transposed on the TensorE (fp32 transpose mode, 128x128 tiles).
Matmuls run in float32r: single-pass fp32 on the PE at full rate
(~1e-4 max rel err vs fp32 reference, measured on hardware).
"""

import numpy as np

import concourse.mybir as mybir
from concourse import bacc
from concourse.tile import TileContext
from concourse.bass_utils import run_bass_kernel_spmd
from concourse.masks import make_identity

E, W, C, D, F = 8, 8, 1024, 512, 2048
P = 128
M_TOT = W * C            # 8192 rows per expert
M_TILE = 512             # rows per m-tile (fp32r moving free dim)
N_MT = M_TOT // M_TILE   # 16
MS = M_TILE // P         # 4 m-subtiles of 128 rows
DC = D // P              # 4 d-chunks
FC = F // P              # 16 f-chunks

F32 = mybir.dt.float32
F32R = mybir.dt.float32r


def _build_nc():
    nc = bacc.Bacc(None, target_bir_lowering=False)

    x = nc.dram_tensor("x", [M_TOT, D], F32, kind="ExternalInput")
    wi = nc.dram_tensor("wi", [D, F], F32, kind="ExternalInput")
    wo = nc.dram_tensor("wo", [F, D], F32, kind="ExternalInput")
    out = nc.dram_tensor("out", [M_TOT, D], F32, kind="ExternalOutput")

    x_v = x.rearrange("(mt ms p) d -> mt p ms d", p=P, ms=MS)
    out_v = out.rearrange("(mt ms p) d -> mt p ms d", p=P, ms=MS)
    wi_v = wi.rearrange("(dc p) f -> p dc f", p=P)
    wo_v = wo.rearrange("(fc p) d -> p fc d", p=P)

    with TileContext(nc) as tc:
        with (
            tc.tile_pool(name="const", bufs=1) as cpool,
            tc.tile_pool(name="xin", bufs=3) as xin_pool,
            tc.tile_pool(name="xt", bufs=2) as xt_pool,
            tc.tile_pool(name="ht", bufs=2) as ht_pool,
            tc.tile_pool(name="osb", bufs=4) as o_pool,
            tc.tile_pool(name="tp_ps", bufs=2, space="PSUM") as tp_psum,
            tc.tile_pool(name="h_ps", bufs=2, space="PSUM") as h_psum,
            tc.tile_pool(name="o_ps", bufs=2, space="PSUM") as o_psum,
        ):
            ident = cpool.tile([P, P], F32)
            make_identity(nc, ident)

            # Weights: DMA fp32 into a staging slot (shares the big "ht"
            # tag so no extra SBUF), then round to fp32r via DVE copy.
            # DMAs split into chunks so they spread across DMA queues.
            wi_sb = cpool.tile([P, DC, F], F32R)
            wo_sb = cpool.tile([P, FC, D], F32R)
            wi_st = ht_pool.tile([P, DC, F], F32, tag="ht")
            wo_st = ht_pool.tile([P, FC, D], F32, tag="ht")
            for dc in range(DC):
                nc.sync.dma_start(wi_st[:, dc], wi_v[:, dc])
                nc.vector.tensor_copy(wi_sb[:, dc], wi_st[:, dc])
            for q in range(4):
                s = slice(q * (FC // 4), (q + 1) * (FC // 4))
                nc.sync.dma_start(wo_st[:, s], wo_v[:, s])
                nc.vector.tensor_copy(wo_sb[:, s], wo_st[:, s])

            def load_x(mt):
                x_nat = xin_pool.tile([P, MS, D], F32)
                nc.sync.dma_start(x_nat, x_v[mt])
                return x_nat

            def transpose_x(x_nat):
                # xT [d, m]: per m-subtile, 4 PE transposes form ONE psum
                # accumulation group in ONE bank (start only on the first,
                # disjoint 128-col regions), drained by ONE wide DVE copy.
                xt = xt_pool.tile([P, DC, M_TILE], F32R)
                for ms in range(MS):
                    tp = tp_psum.tile([P, DC, P], F32)
                    for dc in range(DC):
                        nc.tensor.matmul(
                            tp[:, dc],
                            x_nat[:, ms, dc * P : (dc + 1) * P],
                            ident,
                            is_transpose=True,
                            start=(dc == 0),
                            stop=(dc == DC - 1),
                            skip_group_check=True,
                        )
                    nc.vector.tensor_copy(xt[:, :, ms * P : (ms + 1) * P], tp)
                return xt

            def gemm1(xt):
                # hT[f, m]; two 4-matmul PSUM groups (adjacent banks of one
                # 2-bank tile) drained by a single ACT relu -> fp32r SBUF.
                hT = ht_pool.tile([P, FC, M_TILE], F32R, tag="ht")
                for fc2 in range(FC // 2):
                    hp = h_psum.tile([P, 2, M_TILE], F32)
                    for half in range(2):
                        fc = 2 * fc2 + half
                        for dc in range(DC):
                            nc.tensor.matmul(
                                hp[:, half],
                                wi_sb[:, dc, fc * P : (fc + 1) * P],
                                xt[:, dc, :],
                                start=(dc == 0),
                                stop=(dc == DC - 1),
                            )
                    nc.scalar.activation(
                        hT[:, 2 * fc2 : 2 * fc2 + 2, :],
                        hp,
                        mybir.ActivationFunctionType.Relu,
                    )
                return hT

            def gemm2(mt, hT):
                # out[m, d] per 128-row subtile
                for ms in range(MS):
                    op = o_psum.tile([P, D], F32)
                    for fc in range(FC):
                        nc.tensor.matmul(
                            op,
                            hT[:, fc, ms * P : (ms + 1) * P],
                            wo_sb[:, fc, :],
                            start=(fc == 0),
                            stop=(fc == FC - 1),
                        )
                    o_t = o_pool.tile([P, D], F32)
                    nc.vector.tensor_copy(o_t, op)
                    nc.sync.dma_start(out_v[mt, :, ms, :], o_t)

            # software pipeline: transpose m-tile t+1 between G1(t) and
            # G2(t) so the xt copy latency hides under GEMM2's matmuls.
            xt = transpose_x(load_x(0))
            for mt in range(N_MT):
                hT = gemm1(xt)
                if mt + 1 < N_MT:
                    xt = transpose_x(load_x(mt + 1))
                gemm2(mt, hT)

    nc.finalize()
    return nc


_CACHE = {}


def _get_nc():
    if "nc" not in _CACHE:
        _CACHE["nc"] = _build_nc()
    return _CACHE["nc"]


def _run(x, wi, wo, **spmd_kwargs):
    """x [E, 8192, 512], wi [E, 512, 2048], wo [E, 2048, 512] -> results."""
    nc = _get_nc()
    in_maps = [
        {
            "x": np.ascontiguousarray(x[e]),
            "wi": np.ascontiguousarray(wi[e]),
            "wo": np.ascontiguousarray(wo[e]),
        }
        for e in range(E)
    ]
    return nc, run_bass_kernel_spmd(nc, in_maps, core_ids=list(range(E)), **spmd_kwargs)


def kernel(dispatched_hidden_states, experts_capacity_usage=None, wi=None, wo=None):
    x = np.asarray(dispatched_hidden_states, dtype=np.float32).reshape(E, M_TOT, D)
    wi_ = np.asarray(wi, dtype=np.float32)
    wo_ = np.asarray(wo, dtype=np.float32)
    _, res = _run(x, wi_, wo_)
    out = np.stack([res.results[e]["out"] for e in range(E)])
    return out.reshape(E, W, C, D)



# revision 2
# speedup vs baseline: 1.1276x; 1.1276x over previous
"""Trainium2 Bass kernel for nn_LocalExperts (MoE grouped FFN).

out[e] = relu(x[e] @ wi[e]) @ wo[e]   for e in 0..7

Expert-parallel over 8 NeuronCores: core e computes expert e's FFN.
Per-core work: x [8192, 512], wi [512, 2048], wo [2048, 512]
  GEMM1: hT[f, m] = wi[d, f].T @ xT[d, m]  (accumulate over 4 d-chunks)
  relu (ScalarE) -> hT in SBUF as bf16
  GEMM2: out[m, d] = hT[f, m].T @ wo[f, d] (accumulate over 16 f-chunks)

All inputs are pre-converted to bf16 on the host (max rel err of the
bf16 pipeline vs the fp32 reference is ~3e-3, measured): halves the
input DMA bytes and removes the on-chip weight-cast pass. x is
transposed on the TensorE (bf16 transpose mode, 128x128 tiles, 1
cycle/row vs 1.5 for fp32r). PSUM accumulation is fp32; the final
output is written back as fp32.

DMA issue order puts x tile 0 ahead of the weights so the PE starts
transposing ~10us in instead of waiting ~38us for 8MB of fp32 weights
(all model DMAs share one logical queue, FIFO).
"""

import numpy as np
import ml_dtypes

import concourse.mybir as mybir
from concourse import bacc
from concourse.tile import TileContext
from concourse.bass_utils import run_bass_kernel_spmd
from concourse.masks import make_identity

E, W, C, D, F = 8, 8, 1024, 512, 2048
P = 128
M_TOT = W * C            # 8192 rows per expert
M_TILE = 512             # rows per m-tile (PSUM fp32 bank = 512 cols)
N_MT = M_TOT // M_TILE   # 16
MS = M_TILE // P         # 4 m-subtiles of 128 rows
DC = D // P              # 4 d-chunks
FC = F // P              # 16 f-chunks

F32 = mybir.dt.float32
BF16 = mybir.dt.bfloat16


def _build_nc():
    nc = bacc.Bacc(None, target_bir_lowering=False)

    x = nc.dram_tensor("x", [M_TOT, D], BF16, kind="ExternalInput")
    wi = nc.dram_tensor("wi", [D, F], BF16, kind="ExternalInput")
    wo = nc.dram_tensor("wo", [F, D], BF16, kind="ExternalInput")
    out = nc.dram_tensor("out", [M_TOT, D], F32, kind="ExternalOutput")

    x_v = x.rearrange("(mt ms p) d -> mt p ms d", p=P, ms=MS)
    out_v = out.rearrange("(mt ms p) d -> mt p ms d", p=P, ms=MS)
    wi_v = wi.rearrange("(dc p) f -> p dc f", p=P)
    wo_v = wo.rearrange("(fc p) d -> p fc d", p=P)

    with TileContext(nc) as tc:
        with (
            tc.tile_pool(name="const", bufs=1) as cpool,
            tc.tile_pool(name="xin", bufs=3) as xin_pool,
            tc.tile_pool(name="xt", bufs=2) as xt_pool,
            tc.tile_pool(name="ht", bufs=2) as ht_pool,
            tc.tile_pool(name="osb", bufs=4) as o_pool,
            tc.tile_pool(name="tp_ps", bufs=2, space="PSUM") as tp_psum,
            tc.tile_pool(name="h_ps", bufs=2, space="PSUM") as h_psum,
            tc.tile_pool(name="o_ps", bufs=2, space="PSUM") as o_psum,
        ):
            ident = cpool.tile([P, P], BF16)
            make_identity(nc, ident)

            wi_sb = cpool.tile([P, DC, F], BF16)
            wo_sb = cpool.tile([P, FC, D], BF16)

            def load_x(mt):
                x_nat = xin_pool.tile([P, MS, D], BF16)
                nc.sync.dma_start(x_nat, x_v[mt])
                return x_nat

            # DMA priority order (one FIFO queue): x0 -> wi -> x1 -> wo.
            # x0 unblocks the transposes, wi unblocks GEMM1(0); wo is not
            # needed until GEMM2(0) ~16us in.
            x0 = load_x(0)
            for dc in range(DC):
                nc.sync.dma_start(wi_sb[:, dc], wi_v[:, dc])
            x1 = load_x(1)
            for q in range(4):
                s = slice(q * (FC // 4), (q + 1) * (FC // 4))
                nc.sync.dma_start(wo_sb[:, s], wo_v[:, s])

            def transpose_x(x_nat):
                # xT [d, m]: per m-subtile, 4 PE transposes form ONE psum
                # accumulation group (start only on the first, disjoint
                # 128-col regions), drained by ONE DVE copy (bf16 2x rate).
                xt = xt_pool.tile([P, DC, M_TILE], BF16)
                for ms in range(MS):
                    tp = tp_psum.tile([P, DC, P], BF16)
                    for dc in range(DC):
                        nc.tensor.matmul(
                            tp[:, dc],
                            x_nat[:, ms, dc * P : (dc + 1) * P],
                            ident,
                            is_transpose=True,
                            start=(dc == 0),
                            stop=(dc == DC - 1),
                            skip_group_check=True,
                        )
                    nc.vector.tensor_copy(xt[:, :, ms * P : (ms + 1) * P], tp)
                return xt

            def gemm1(xt):
                # hT[f, m]; two 4-matmul PSUM groups (adjacent banks of one
                # 2-bank tile) drained by a single ACT relu -> bf16 SBUF.
                hT = ht_pool.tile([P, FC, M_TILE], BF16)
                for fc2 in range(FC // 2):
                    hp = h_psum.tile([P, 2, M_TILE], F32)
                    for half in range(2):
                        fc = 2 * fc2 + half
                        for dc in range(DC):
                            nc.tensor.matmul(
                                hp[:, half],
                                wi_sb[:, dc, fc * P : (fc + 1) * P],
                                xt[:, dc, :],
                                start=(dc == 0),
                                stop=(dc == DC - 1),
                            )
                    nc.scalar.activation(
                        hT[:, 2 * fc2 : 2 * fc2 + 2, :],
                        hp,
                        mybir.ActivationFunctionType.Relu,
                    )
                return hT

            def gemm2(mt, hT):
                # out[m, d] per 128-row subtile; fc ascending so the first
                # 14 matmuls only need relu chunks that finished long ago.
                for ms in range(MS):
                    op = o_psum.tile([P, D], F32)
                    for fc in range(FC):
                        nc.tensor.matmul(
                            op,
                            hT[:, fc, ms * P : (ms + 1) * P],
                            wo_sb[:, fc, :],
                            start=(fc == 0),
                            stop=(fc == FC - 1),
                        )
                    o_t = o_pool.tile([P, D], F32)
                    nc.vector.tensor_copy(o_t, op)
                    nc.sync.dma_start(out_v[mt, :, ms, :], o_t)

            # software pipeline: transpose m-tile t+1 between G1(t) and
            # G2(t) so the xt copy latency hides under GEMM2's matmuls.
            xt = transpose_x(x0)
            for mt in range(N_MT):
                hT = gemm1(xt)
                if mt + 1 < N_MT:
                    xt = transpose_x(x1 if mt == 0 else load_x(mt + 1))
                gemm2(mt, hT)

    nc.finalize()
    return nc


_CACHE = {}


def _get_nc():
    if "nc" not in _CACHE:
        _CACHE["nc"] = _build_nc()
    return _CACHE["nc"]


def _run(x, wi, wo, **spmd_kwargs):
    """x [E, 8192, 512], wi [E, 512, 2048], wo [E, 2048, 512] -> results."""
    nc = _get_nc()
    x_bf = np.asarray(x, dtype=np.float32).astype(ml_dtypes.bfloat16)
    wi_bf = np.asarray(wi, dtype=np.float32).astype(ml_dtypes.bfloat16)
    wo_bf = np.asarray(wo, dtype=np.float32).astype(ml_dtypes.bfloat16)
    in_maps = [
        {
            "x": np.ascontiguousarray(x_bf[e]),
            "wi": np.ascontiguousarray(wi_bf[e]),
            "wo": np.ascontiguousarray(wo_bf[e]),
        }
        for e in range(E)
    ]
    return nc, run_bass_kernel_spmd(nc, in_maps, core_ids=list(range(E)), **spmd_kwargs)


def kernel(dispatched_hidden_states, experts_capacity_usage=None, wi=None, wo=None):
    x = np.asarray(dispatched_hidden_states, dtype=np.float32).reshape(E, M_TOT, D)
    wi_ = np.asarray(wi, dtype=np.float32)
    wo_ = np.asarray(wo, dtype=np.float32)
    _, res = _run(x, wi_, wo_)
    out = np.stack([res.results[e]["out"] for e in range(E)])
    return out.reshape(E, W, C, D)
